# revision 8
# baseline (speedup 1.0000x reference)
"""Trainium2 Bass kernel for nn_MultiHeadAttention_48782238548272.

Model (reference):
    Q/K/V = lstm @ W{q,k,v}.T + b        -> [B, H, S, Hd]
    energy = QK^T * (1/sqrt(Hd)); mask; attn = softmax(energy)  [B,H,S,S]
    out = (attn @ V) -> merge heads -> @ Wo.T + bo -> sum over S
    returns (weighted [B, D], attn [B, H, S, S])

Shapes: B=4, S=2048, D=1024, H=16, Hd=64.

Sharding (8 cores): core c handles batch b=c//2 and head-group g=c%2
(8 heads). Each core computes its Q/K/V projections (tensor-parallel
column shard of the weights), the 8 heads' attention matrices
(energy -> exp -> normalize) and writes the 134MB attn shard straight
to HBM.  Because `weighted` = (sum_q ctx) @ Wo.T + S*bo and
sum_q ctx = sum_k colsum(attn)[k] * V[k,:], the device additionally
emits per-head attn column-sums a[h,k] (via ones^T @ attn on the PE)
and V^T; the final (tiny, ~17 MFLOP) a@V and @Wo.T contractions are
folded into the host-side unshard step.

Device layout notes:
  - x is fed pre-transposed (xT [D, S]) so every matmul contracts over
    the hidden dim on partitions.
  - Q/K/V are produced in head-transposed layout ([d_pair=128, S]) per
    head-pair, so energy matmuls contract d=64 on partitions with
    row-tiled (64-row) PE mode, two heads concurrently (T0/T8).
  - softmax needs no max-subtraction: energy*scale is ~N(0,1), exp is
    safe in fp32.  exp runs on ACT with accum_out giving row-sums for
    free; normalization is one DVE tensor_scalar per tile (2x mode).
  - attn column-sums accumulate in PSUM across all 16 q-tiles using
    column-tiled (32-col) matmuls so 4 k-chunks share one PSUM bank.
  - Projection/energy matmul inputs are fp16 (full-rate PE mode; fp32
    is 4x slower, and fp32/fp32r self-loading matmuls can carry only a
    single semaphore wait which breaks Tile's scheduling).  The attn
    column-sum matmuls read the f32 attn tiles directly (4 cyc/row,
    hidden by col-tiling concurrency); attn itself stays exact f32.
"""

import numpy as np

B_, S_, D_, H_, HD_ = 4, 2048, 1024, 16, 64
G_ = 512          # columns of the projection handled per core (8 heads)
SCALE = 0.125     # 1/sqrt(64)
NCORES = 8

_CACHE = {}


def _build_nc(loop=1):
    import concourse.bacc as bacc
    import concourse.mybir as mybir
    import concourse.tile as tile

    f32 = mybir.dt.float32
    f16 = mybir.dt.float16
    AF = mybir.ActivationFunctionType

    nc = bacc.Bacc("TRN2", target_bir_lowering=False, debug=False)

    xT = nc.dram_tensor("xT", [D_, S_], f16, kind="ExternalInput")
    wqT = nc.dram_tensor("wqT", [D_, G_], f16, kind="ExternalInput")
    wkT = nc.dram_tensor("wkT", [D_, G_], f16, kind="ExternalInput")
    wvT = nc.dram_tensor("wvT", [D_, G_], f16, kind="ExternalInput")
    bq = nc.dram_tensor("bq", [G_], f32, kind="ExternalInput")
    bk = nc.dram_tensor("bk", [G_], f32, kind="ExternalInput")
    bv = nc.dram_tensor("bv", [G_], f32, kind="ExternalInput")
    attn_o = nc.dram_tensor("attn_o", [8, S_, S_], f32, kind="ExternalOutput")
    a_o = nc.dram_tensor("a_o", [8, S_], f32, kind="ExternalOutput")
    vt_o = nc.dram_tensor("vt_o", [4, 128, S_], f32, kind="ExternalOutput")

    a_o_r = a_o[:].rearrange("h (j f) -> h j f", j=4)

    with tile.TileContext(nc) as tc, \
            tc.tile_pool(name="constp", bufs=1) as constp, \
            tc.tile_pool(name="wp", bufs=2) as wp, \
            tc.tile_pool(name="qkvp", bufs=2) as qkvp, \
            tc.tile_pool(name="attnp", bufs=4) as attnp, \
            tc.tile_pool(name="smallp", bufs=4) as smallp, \
            tc.tile_pool(name="ahp", bufs=2) as ahp, \
            tc.tile_pool(name="mmp", bufs=3, space="PSUM") as mmp, \
            tc.tile_pool(name="csp", bufs=2, space="PSUM") as csp:

        xT_sb = constp.tile([128, 8, S_], f16, name="xT_sb")
        nc.sync.dma_start(xT_sb[:], xT[:].rearrange("(kt p) s -> p kt s", p=128))

        ones_col = constp.tile([128, 1], f32, name="ones_col")
        nc.vector.memset(ones_col[:], 1.0)

        bias_sb = {}
        for nm, bt in (("q", bq), ("k", bk), ("v", bv)):
            t = constp.tile([128, 4], f32, name=f"b{nm}_sb")
            nc.sync.dma_start(t[:], bt[:].rearrange("(hp p) -> p hp", p=128))
            bias_sb[nm] = t

        import contextlib
        loop_cm = tc.For_i(0, loop, 1) if loop > 1 else contextlib.nullcontext()
        with loop_cm:
            _body(nc, tc, constp, wp, qkvp, attnp, smallp, ahp, mmp, csp,
                  xT_sb, ones_col, bias_sb,
                  wqT, wkT, wvT, attn_o, a_o_r, vt_o, f32, f16, AF)

    nc.compile()
    return nc


def _body(nc, tc, constp, wp, qkvp, attnp, smallp, ahp, mmp, csp,
          xT_sb, ones_col, bias_sb,
          wqT, wkT, wvT, attn_o, a_o_r, vt_o, f32, f16, AF):
        for hp in range(4):
            w_sb = {}
            for nm, wt in (("q", wqT), ("k", wkT), ("v", wvT)):
                t = wp.tile([128, 8, 128], f16, name=f"w{nm}_sb", tag=f"w{nm}")
                nc.sync.dma_start(
                    t[:],
                    wt[:].rearrange("(kt p) m -> p kt m", p=128)[
                        :, :, hp * 128:(hp + 1) * 128
                    ],
                )
                w_sb[nm] = t

            # --- projections: {Q,K,V}t_pair [128, S] (d on partitions) ---
            qkv = {}
            for nm in ("q", "k", "v"):
                dt_o = f32 if nm == "v" else f16
                dst = qkvp.tile([128, S_], dt_o, name=f"{nm}t", tag=f"{nm}t")
                for half in range(2):
                    ps = mmp.tile([128, 1024], f32, name="proj_ps", tag="mm")
                    for nck in range(2):
                        c0 = nck * 512
                        s0 = half * 1024 + c0
                        for kt in range(8):
                            nc.tensor.matmul(
                                ps[:, c0:c0 + 512],
                                w_sb[nm][:, kt, :],
                                xT_sb[:, kt, s0:s0 + 512],
                                start=(kt == 0),
                                stop=(kt == 7),
                            )
                    nc.vector.tensor_scalar_add(
                        dst[:, half * 1024:(half + 1) * 1024],
                        ps[:],
                        bias_sb[nm][:, hp:hp + 1],
                    )
                qkv[nm] = dst

            nc.sync.dma_start(vt_o[hp], qkv["v"][:])

            # --- attention for the two heads of this pair ---
            cs_ps = [
                csp.tile([128, 512], f32, name=f"cs{h}", tag="cs")
                for h in range(2)
            ]
            for qt in range(16):
                q0 = qt * 128
                eps = {}
                for half in range(2):
                    for h in range(2):
                        eps[(h, half)] = mmp.tile(
                            [128, 1024], f32, name="e_ps", tag="mm")
                    for ncx in range(2):
                        for h in range(2):
                            r0 = h * 64
                            k0 = half * 1024 + ncx * 512
                            nc.tensor.matmul(
                                eps[(h, half)][:, ncx * 512:(ncx + 1) * 512],
                                qkv["q"][r0:r0 + 64, q0:q0 + 128],
                                qkv["k"][r0:r0 + 64, k0:k0 + 512],
                                start=True,
                                stop=True,
                                tile_position=(r0, 0),
                            )

                acc = smallp.tile([128, 4], f32, name="acc", tag="acc")
                at2 = attnp.tile([128, 2 * S_], f32, name="attn_t", tag="attn")
                for h in range(2):
                    for half in range(2):
                        nc.scalar.activation(
                            at2[:, h * S_ + half * 1024:h * S_ + (half + 1) * 1024],
                            eps[(h, half)][:],
                            AF.Exp,
                            scale=SCALE,
                            accum_out=acc[:, half * 2 + h:half * 2 + h + 1],
                        )

                s2 = smallp.tile([128, 2], f32, name="s2", tag="s2")
                nc.vector.tensor_add(s2[:], acc[:, 0:2], acc[:, 2:4])
                r2 = smallp.tile([128, 2], f32, name="r2", tag="r2")
                nc.vector.reciprocal(r2[:], s2[:])

                for h in range(2):
                    nc.vector.tensor_scalar_mul(
                        at2[:, h * S_:(h + 1) * S_],
                        at2[:, h * S_:(h + 1) * S_],
                        r2[:, h:h + 1],
                    )
                    for j in range(4):
                        nc.tensor.matmul(
                            cs_ps[h][32 * j:32 * j + 1, :],
                            ones_col[:],
                            at2[:, h * S_ + j * 512:h * S_ + (j + 1) * 512],
                            start=(qt == 0),
                            stop=(qt == 15),
                            tile_position=(0, 32 * j),
                        )
                nc.sync.dma_start(
                    attn_o[hp * 2:hp * 2 + 2, q0:q0 + 128, :].rearrange(
                        "h p k -> p h k"
                    ),
                    at2[:].rearrange("p (h k) -> p h k", h=2),
                )

            for h in range(2):
                ah = ahp.tile([128, 512], f32, name="ah", tag="ah")
                for j in range(4):
                    nc.vector.tensor_copy(
                        ah[32 * j:32 * j + 1, :], cs_ps[h][32 * j:32 * j + 1, :]
                    )
                for j in range(4):
                    nc.sync.dma_start(
                        a_o_r[hp * 2 + h][j:j + 1, :], ah[32 * j:32 * j + 1, :]
                    )


def _get_nc():
    if "nc" not in _CACHE:
        _CACHE["nc"] = _build_nc()
    return _CACHE["nc"]


def _numpy_reference(lstm_output, mask, Wq, bq, Wk, bk, Wv, bv, Wo, bo):
    """Exact numpy fallback (used only if mask is not all-ones)."""
    B, S, D = lstm_output.shape
    H, Hd = H_, HD_

    def proj(W, b):
        y = lstm_output @ W.T + b
        return y.reshape(B, S, H, Hd).transpose(0, 2, 1, 3)

    Q, K, V = proj(Wq, bq), proj(Wk, bk), proj(Wv, bv)
    energy = np.einsum("bhqd,bhkd->bhqk", Q, K) * np.float32(1.0 / np.sqrt(Hd))
    m = mask[:, None, None, :]
    energy = np.where(m == 0, np.float32(-10000.0), energy).astype(np.float32)
    e = np.exp(energy - energy.max(axis=-1, keepdims=True))
    attn = e / e.sum(axis=-1, keepdims=True)
    out = np.einsum("bhqk,bhkd->bhqd", attn, V)
    out = out.transpose(0, 2, 1, 3).reshape(B, S, D)
    out = out @ Wo.T + bo
    weighted = out.sum(axis=1)
    return weighted.astype(np.float32), attn.astype(np.float32)


def kernel(lstm_output, mask, Wq, bq, Wk, bk, Wv, bv, Wo, bo):
    lstm_output = np.ascontiguousarray(np.asarray(lstm_output, dtype=np.float32))
    mask = np.asarray(mask)
    Wq, Wk, Wv, Wo = (np.asarray(w, dtype=np.float32) for w in (Wq, Wk, Wv, Wo))
    bq, bk, bv, bo = (np.asarray(b, dtype=np.float32) for b in (bq, bk, bv, bo))

    if not np.all(mask == 1):
        return _numpy_reference(
            lstm_output, mask, Wq, bq, Wk, bk, Wv, bv, Wo, bo
        )

    nc = _get_nc()

    xTs = [np.ascontiguousarray(lstm_output[b].T.astype(np.float16)) for b in range(B_)]
    wqTs = [np.ascontiguousarray(Wq[g * G_:(g + 1) * G_, :].T.astype(np.float16)) for g in range(2)]
    wkTs = [np.ascontiguousarray(Wk[g * G_:(g + 1) * G_, :].T.astype(np.float16)) for g in range(2)]
    wvTs = [np.ascontiguousarray(Wv[g * G_:(g + 1) * G_, :].T.astype(np.float16)) for g in range(2)]

    in_maps = []
    for c in range(NCORES):
        b, g = c // 2, c % 2
        in_maps.append({
            "xT": xTs[b],
            "wqT": wqTs[g],
            "wkT": wkTs[g],
            "wvT": wvTs[g],
            "bq": np.ascontiguousarray(bq[g * G_:(g + 1) * G_]),
            "bk": np.ascontiguousarray(bk[g * G_:(g + 1) * G_]),
            "bv": np.ascontiguousarray(bv[g * G_:(g + 1) * G_]),
        })

    from concourse import bass_utils

    res = bass_utils.run_bass_kernel_spmd(
        nc, in_maps, core_ids=list(range(NCORES))
    )
    outs = res.results

    attn = np.empty((B_, H_, S_, S_), np.float32)
    a_all = np.empty((B_, H_, S_), np.float32)
    vt_all = np.empty((B_, 2, 4, 128, S_), np.float32)
    for c in range(NCORES):
        b, g = c // 2, c % 2
        attn[b, g * 8:(g + 1) * 8] = outs[c]["attn_o"]
        a_all[b, g * 8:(g + 1) * 8] = outs[c]["a_o"]
        vt_all[b, g] = outs[c]["vt_o"]

    # Host-side unshard tail (~17 MFLOP):
    # wsum[b, h, d] = sum_k a[b, h, k] * V[b, k, h*64+d]
    # vt_all[b, g, hp, head*64+d, k] holds V^T for head h = g*8 + 2*hp + head
    vt = vt_all.reshape(B_, 2, 4, 2, 64, S_)          # [b, g, hp, head, d, k]
    vt = vt.reshape(B_, H_, 64, S_)                   # [b, h, d, k]
    wsum = np.einsum("bhdk,bhk->bhd", vt, a_all)      # [b, h, d]
    weighted = wsum.reshape(B_, D_) @ Wo.T + np.float32(S_) * bo
    return weighted.astype(np.float32), attn


if __name__ == "__main__":
    # smoke-build only
    nco = _get_nc()
    print("built ok:", nco)


# revision 9
# speedup vs baseline: 1.1751x; 1.1751x over previous
"""Trainium2 Bass kernel for nn_MultiHeadAttention_48782238548272.

Model (reference):
    Q/K/V = lstm @ W{q,k,v}.T + b        -> [B, H, S, Hd]
    energy = QK^T * (1/sqrt(Hd)); mask; attn = softmax(energy)  [B,H,S,S]
    out = (attn @ V) -> merge heads -> @ Wo.T + bo -> sum over S
    returns (weighted [B, D], attn [B, H, S, S])

Shapes: B=4, S=2048, D=1024, H=16, Hd=64.

Sharding (8 cores): core c handles batch b=c//2 and head-group g=c%2
(8 heads). Each core computes its Q/K/V projections (tensor-parallel
column shard of the weights), the 8 heads' attention matrices
(energy -> exp -> normalize) and writes the 134MB attn shard straight
to HBM.  Because `weighted` = (sum_q ctx) @ Wo.T + S*bo and
sum_q ctx = sum_k colsum(attn)[k] * V[k,:], the device additionally
emits per-head attn column-sums a[h,k] (via ones^T @ attn on the PE)
and V^T; the final (tiny, ~17 MFLOP) a@V and @Wo.T contractions are
folded into the host-side unshard step.

Device layout notes:
  - x is fed pre-transposed (xT [D, S]) so every matmul contracts over
    the hidden dim on partitions.
  - Q/K/V are produced in head-transposed layout ([d_pair=128, S]) per
    head-pair, so energy matmuls contract d=64 on partitions with
    row-tiled (64-row) PE mode, two heads concurrently (T0/T8).
  - softmax needs no max-subtraction: energy*scale is ~N(0,1), exp is
    safe in fp32.  exp runs on ACT with accum_out giving row-sums for
    free; normalization is one DVE tensor_scalar per tile (2x mode).
  - attn column-sums accumulate in PSUM across all 16 q-tiles using
    column-tiled (32-col) matmuls so 4 k-chunks share one PSUM bank.
  - Projection/energy matmul inputs are fp16 (full-rate PE mode; fp32
    is 4x slower, and fp32/fp32r self-loading matmuls can carry only a
    single semaphore wait which breaks Tile's scheduling).  The attn
    column-sum matmuls read the f32 attn tiles directly (4 cyc/row,
    hidden by col-tiling concurrency); attn itself stays exact f32.
"""

import numpy as np

B_, S_, D_, H_, HD_ = 4, 2048, 1024, 16, 64
G_ = 512          # columns of the projection handled per core (8 heads)
SCALE = 0.125     # 1/sqrt(64)
NCORES = 8

_CACHE = {}


def _build_nc(loop=1):
    import concourse.bacc as bacc
    import concourse.mybir as mybir
    import concourse.tile as tile

    f32 = mybir.dt.float32
    f16 = mybir.dt.float16
    AF = mybir.ActivationFunctionType

    nc = bacc.Bacc("TRN2", target_bir_lowering=False, debug=False)

    xT = nc.dram_tensor("xT", [D_, S_], f16, kind="ExternalInput")
    wqT = nc.dram_tensor("wqT", [D_, G_], f16, kind="ExternalInput")
    wkT = nc.dram_tensor("wkT", [D_, G_], f16, kind="ExternalInput")
    wvT = nc.dram_tensor("wvT", [D_, G_], f16, kind="ExternalInput")
    bq = nc.dram_tensor("bq", [G_], f32, kind="ExternalInput")
    bk = nc.dram_tensor("bk", [G_], f32, kind="ExternalInput")
    bv = nc.dram_tensor("bv", [G_], f32, kind="ExternalInput")
    attn_o = nc.dram_tensor("attn_o", [8, S_, S_], f32, kind="ExternalOutput")
    a_o = nc.dram_tensor("a_o", [8, S_], f32, kind="ExternalOutput")
    vt_o = nc.dram_tensor("vt_o", [4, 128, S_], f32, kind="ExternalOutput")

    a_o_r = a_o[:].rearrange("h (j f) -> h j f", j=4)

    with tile.TileContext(nc) as tc, \
            tc.tile_pool(name="constp", bufs=1) as constp, \
            tc.tile_pool(name="wp", bufs=2) as wp, \
            tc.tile_pool(name="qkvp", bufs=2) as qkvp, \
            tc.tile_pool(name="attnp", bufs=4) as attnp, \
            tc.tile_pool(name="smallp", bufs=4) as smallp, \
            tc.tile_pool(name="ahp", bufs=2) as ahp, \
            tc.tile_pool(name="mmp", bufs=3, space="PSUM") as mmp, \
            tc.tile_pool(name="csp", bufs=2, space="PSUM") as csp:

        xT_sb = constp.tile([128, 8, S_], f16, name="xT_sb")
        nc.sync.dma_start(xT_sb[:], xT[:].rearrange("(kt p) s -> p kt s", p=128))

        ones_col = constp.tile([128, 1], f32, name="ones_col")
        nc.vector.memset(ones_col[:], 1.0)

        bias_sb = {}
        for nm, bt in (("q", bq), ("k", bk), ("v", bv)):
            t = constp.tile([128, 4], f32, name=f"b{nm}_sb")
            nc.sync.dma_start(t[:], bt[:].rearrange("(hp p) -> p hp", p=128))
            bias_sb[nm] = t

        import contextlib
        loop_cm = tc.For_i(0, loop, 1) if loop > 1 else contextlib.nullcontext()
        with loop_cm:
            _body(nc, tc, constp, wp, qkvp, attnp, smallp, ahp, mmp, csp,
                  xT_sb, ones_col, bias_sb,
                  wqT, wkT, wvT, attn_o, a_o_r, vt_o, f32, f16, AF)

    nc.compile()
    return nc


def _body(nc, tc, constp, wp, qkvp, attnp, smallp, ahp, mmp, csp,
          xT_sb, ones_col, bias_sb,
          wqT, wkT, wvT, attn_o, a_o_r, vt_o, f32, f16, AF):
        for hp in range(4):
            w_sb = {}
            for nm, wt in (("q", wqT), ("k", wkT), ("v", wvT)):
                t = wp.tile([128, 8, 128], f16, name=f"w{nm}_sb", tag=f"w{nm}")
                nc.sync.dma_start(
                    t[:],
                    wt[:].rearrange("(kt p) m -> p kt m", p=128)[
                        :, :, hp * 128:(hp + 1) * 128
                    ],
                )
                w_sb[nm] = t

            # --- projections: {Q,K,V}t_pair [128, S] (d on partitions) ---
            qkv = {}
            for nm in ("q", "k", "v"):
                dt_o = f32 if nm == "v" else f16
                dst = qkvp.tile([128, S_], dt_o, name=f"{nm}t", tag=f"{nm}t")
                for half in range(2):
                    ps = mmp.tile([128, 1024], f32, name="proj_ps", tag="mm")
                    for nck in range(2):
                        c0 = nck * 512
                        s0 = half * 1024 + c0
                        for kt in range(8):
                            nc.tensor.matmul(
                                ps[:, c0:c0 + 512],
                                w_sb[nm][:, kt, :],
                                xT_sb[:, kt, s0:s0 + 512],
                                start=(kt == 0),
                                stop=(kt == 7),
                            )
                    nc.vector.tensor_scalar_add(
                        dst[:, half * 1024:(half + 1) * 1024],
                        ps[:],
                        bias_sb[nm][:, hp:hp + 1],
                    )
                qkv[nm] = dst

            nc.sync.dma_start(vt_o[hp], qkv["v"][:])

            # --- attention for the two heads of this pair ---
            cs_ps = [
                csp.tile([128, 512], f32, name=f"cs{h}", tag="cs")
                for h in range(2)
            ]

            def emit_colsum(qt, at2):
                for h in range(2):
                    for j in range(4):
                        nc.tensor.matmul(
                            cs_ps[h][32 * j:32 * j + 1, :],
                            ones_col[:],
                            at2[:, h * S_ + j * 512:h * S_ + (j + 1) * 512],
                            start=(qt == 0),
                            stop=(qt == 15),
                            tile_position=(0, 32 * j),
                        )

            pending = []
            for qt in range(16):
                q0 = qt * 128
                eps = {}
                for half in range(2):
                    for h in range(2):
                        eps[(h, half)] = mmp.tile(
                            [128, 1024], f32, name="e_ps", tag="mm")
                    for ncx in range(2):
                        for h in range(2):
                            r0 = h * 64
                            k0 = half * 1024 + ncx * 512
                            nc.tensor.matmul(
                                eps[(h, half)][:, ncx * 512:(ncx + 1) * 512],
                                qkv["q"][r0:r0 + 64, q0:q0 + 128],
                                qkv["k"][r0:r0 + 64, k0:k0 + 512],
                                start=True,
                                stop=True,
                                tile_position=(r0, 0),
                            )

                acc = smallp.tile([128, 4], f32, name="acc", tag="acc")
                at2 = attnp.tile([128, 2 * S_], f32, name="attn_t", tag="attn")
                for h in range(2):
                    for half in range(2):
                        nc.scalar.activation(
                            at2[:, h * S_ + half * 1024:h * S_ + (half + 1) * 1024],
                            eps[(h, half)][:],
                            AF.Exp,
                            scale=SCALE,
                            accum_out=acc[:, half * 2 + h:half * 2 + h + 1],
                        )

                s2 = smallp.tile([128, 2], f32, name="s2", tag="s2")
                nc.vector.tensor_add(s2[:], acc[:, 0:2], acc[:, 2:4])
                r2 = smallp.tile([128, 2], f32, name="r2", tag="r2")
                nc.vector.reciprocal(r2[:], s2[:])

                for h in range(2):
                    nc.vector.tensor_scalar_mul(
                        at2[:, h * S_:(h + 1) * S_],
                        at2[:, h * S_:(h + 1) * S_],
                        r2[:, h:h + 1],
                    )
                nc.sync.dma_start(
                    attn_o[hp * 2:hp * 2 + 2, q0:q0 + 128, :].rearrange(
                        "h p k -> p h k"
                    ),
                    at2[:].rearrange("p (h k) -> p h k", h=2),
                )
                # colsum lags one q-tile so the PE never stalls on the
                # exp->rowsum->reciprocal->normalize chain of this tile.
                pending.append((qt, at2))
                if len(pending) > 1:
                    emit_colsum(*pending.pop(0))
            while pending:
                emit_colsum(*pending.pop(0))

            for h in range(2):
                ah = ahp.tile([128, 512], f32, name="ah", tag="ah")
                for j in range(4):
                    nc.vector.tensor_copy(
                        ah[32 * j:32 * j + 1, :], cs_ps[h][32 * j:32 * j + 1, :]
                    )
                for j in range(4):
                    nc.sync.dma_start(
                        a_o_r[hp * 2 + h][j:j + 1, :], ah[32 * j:32 * j + 1, :]
                    )


def _get_nc():
    if "nc" not in _CACHE:
        _CACHE["nc"] = _build_nc()
    return _CACHE["nc"]


def _numpy_reference(lstm_output, mask, Wq, bq, Wk, bk, Wv, bv, Wo, bo):
    """Exact numpy fallback (used only if mask is not all-ones)."""
    B, S, D = lstm_output.shape
    H, Hd = H_, HD_

    def proj(W, b):
        y = lstm_output @ W.T + b
        return y.reshape(B, S, H, Hd).transpose(0, 2, 1, 3)

    Q, K, V = proj(Wq, bq), proj(Wk, bk), proj(Wv, bv)
    energy = np.einsum("bhqd,bhkd->bhqk", Q, K) * np.float32(1.0 / np.sqrt(Hd))
    m = mask[:, None, None, :]
    energy = np.where(m == 0, np.float32(-10000.0), energy).astype(np.float32)
    e = np.exp(energy - energy.max(axis=-1, keepdims=True))
    attn = e / e.sum(axis=-1, keepdims=True)
    out = np.einsum("bhqk,bhkd->bhqd", attn, V)
    out = out.transpose(0, 2, 1, 3).reshape(B, S, D)
    out = out @ Wo.T + bo
    weighted = out.sum(axis=1)
    return weighted.astype(np.float32), attn.astype(np.float32)


def kernel(lstm_output, mask, Wq, bq, Wk, bk, Wv, bv, Wo, bo):
    lstm_output = np.ascontiguousarray(np.asarray(lstm_output, dtype=np.float32))
    mask = np.asarray(mask)
    Wq, Wk, Wv, Wo = (np.asarray(w, dtype=np.float32) for w in (Wq, Wk, Wv, Wo))
    bq, bk, bv, bo = (np.asarray(b, dtype=np.float32) for b in (bq, bk, bv, bo))

    if not np.all(mask == 1):
        return _numpy_reference(
            lstm_output, mask, Wq, bq, Wk, bk, Wv, bv, Wo, bo
        )

    nc = _get_nc()

    xTs = [np.ascontiguousarray(lstm_output[b].T.astype(np.float16)) for b in range(B_)]
    wqTs = [np.ascontiguousarray(Wq[g * G_:(g + 1) * G_, :].T.astype(np.float16)) for g in range(2)]
    wkTs = [np.ascontiguousarray(Wk[g * G_:(g + 1) * G_, :].T.astype(np.float16)) for g in range(2)]
    wvTs = [np.ascontiguousarray(Wv[g * G_:(g + 1) * G_, :].T.astype(np.float16)) for g in range(2)]

    in_maps = []
    for c in range(NCORES):
        b, g = c // 2, c % 2
        in_maps.append({
            "xT": xTs[b],
            "wqT": wqTs[g],
            "wkT": wkTs[g],
            "wvT": wvTs[g],
            "bq": np.ascontiguousarray(bq[g * G_:(g + 1) * G_]),
            "bk": np.ascontiguousarray(bk[g * G_:(g + 1) * G_]),
            "bv": np.ascontiguousarray(bv[g * G_:(g + 1) * G_]),
        })

    from concourse import bass_utils

    res = bass_utils.run_bass_kernel_spmd(
        nc, in_maps, core_ids=list(range(NCORES))
    )
    outs = res.results

    attn = np.empty((B_, H_, S_, S_), np.float32)
    a_all = np.empty((B_, H_, S_), np.float32)
    vt_all = np.empty((B_, 2, 4, 128, S_), np.float32)
    for c in range(NCORES):
        b, g = c // 2, c % 2
        attn[b, g * 8:(g + 1) * 8] = outs[c]["attn_o"]
        a_all[b, g * 8:(g + 1) * 8] = outs[c]["a_o"]
        vt_all[b, g] = outs[c]["vt_o"]

    # Host-side unshard tail (~17 MFLOP):
    # wsum[b, h, d] = sum_k a[b, h, k] * V[b, k, h*64+d]
    # vt_all[b, g, hp, head*64+d, k] holds V^T for head h = g*8 + 2*hp + head
    vt = vt_all.reshape(B_, 2, 4, 2, 64, S_)          # [b, g, hp, head, d, k]
    vt = vt.reshape(B_, H_, 64, S_)                   # [b, h, d, k]
    wsum = np.einsum("bhdk,bhk->bhd", vt, a_all)      # [b, h, d]
    weighted = wsum.reshape(B_, D_) @ Wo.T + np.float32(S_) * bo
    return weighted.astype(np.float32), attn


if __name__ == "__main__":
    # smoke-build only
    nco = _get_nc()
    print("built ok:", nco)


# revision 10
# speedup vs baseline: 1.2634x; 1.0751x over previous
"""Trainium2 Bass kernel for nn_MultiHeadAttention_48782238548272.

Model (reference):
    Q/K/V = lstm @ W{q,k,v}.T + b        -> [B, H, S, Hd]
    energy = QK^T * (1/sqrt(Hd)); mask; attn = softmax(energy)  [B,H,S,S]
    out = (attn @ V) -> merge heads -> @ Wo.T + bo -> sum over S
    returns (weighted [B, D], attn [B, H, S, S])

Shapes: B=4, S=2048, D=1024, H=16, Hd=64.

Sharding (8 cores): core c handles batch b=c//2 and head-group g=c%2
(8 heads). Each core computes its Q/K/V projections (tensor-parallel
column shard of the weights), the 8 heads' attention matrices
(energy -> exp -> normalize) and writes the 134MB attn shard straight
to HBM.  Because `weighted` = (sum_q ctx) @ Wo.T + S*bo and
sum_q ctx = sum_k colsum(attn)[k] * V[k,:], the device additionally
emits per-head attn column-sums a[h,k] (via ones^T @ attn on the PE)
and V^T; the final (tiny, ~17 MFLOP) a@V and @Wo.T contractions are
folded into the host-side unshard step.

Device layout notes:
  - x is fed pre-transposed (xT [D, S]) so every matmul contracts over
    the hidden dim on partitions.
  - Q/K/V are produced in head-transposed layout ([d_pair=128, S]) per
    head-pair, so energy matmuls contract d=64 on partitions with
    row-tiled (64-row) PE mode, two heads concurrently (T0/T8).
  - softmax needs no max-subtraction: energy*scale is ~N(0,1), exp is
    safe in fp32.  exp runs on ACT with accum_out giving row-sums for
    free; normalization is one DVE tensor_scalar per tile (2x mode).
  - attn column-sums accumulate in PSUM across all 16 q-tiles using
    column-tiled (32-col) matmuls so 4 k-chunks share one PSUM bank.
  - Projection/energy matmul inputs are fp16 (full-rate PE mode; fp32
    is 4x slower, and fp32/fp32r self-loading matmuls can carry only a
    single semaphore wait which breaks Tile's scheduling).  The attn
    column-sum matmuls read the f32 attn tiles directly (4 cyc/row,
    hidden by col-tiling concurrency); attn itself stays exact f32.
"""

import numpy as np

B_, S_, D_, H_, HD_ = 4, 2048, 1024, 16, 64
G_ = 512          # columns of the projection handled per core (8 heads)
SCALE = 0.125     # 1/sqrt(64)
NCORES = 8

_CACHE = {}


def _build_nc(loop=1):
    import concourse.bacc as bacc
    import concourse.mybir as mybir
    import concourse.tile as tile

    f32 = mybir.dt.float32
    f16 = mybir.dt.float16
    AF = mybir.ActivationFunctionType

    nc = bacc.Bacc("TRN2", target_bir_lowering=False, debug=False)

    xT = nc.dram_tensor("xT", [D_, S_], f16, kind="ExternalInput")
    wqT = nc.dram_tensor("wqT", [D_, G_], f16, kind="ExternalInput")
    wkT = nc.dram_tensor("wkT", [D_, G_], f16, kind="ExternalInput")
    wvT = nc.dram_tensor("wvT", [D_, G_], f16, kind="ExternalInput")
    bq = nc.dram_tensor("bq", [G_], f32, kind="ExternalInput")
    bk = nc.dram_tensor("bk", [G_], f32, kind="ExternalInput")
    bv = nc.dram_tensor("bv", [G_], f32, kind="ExternalInput")
    attn_o = nc.dram_tensor("attn_o", [8, S_, S_], f32, kind="ExternalOutput")
    a_o = nc.dram_tensor("a_o", [8, S_], f32, kind="ExternalOutput")
    vt_o = nc.dram_tensor("vt_o", [4, 128, S_], f32, kind="ExternalOutput")

    a_o_r = a_o[:].rearrange("h (j f) -> h j f", j=4)

    with tile.TileContext(nc) as tc, \
            tc.tile_pool(name="constp", bufs=1) as constp, \
            tc.tile_pool(name="wp", bufs=2) as wp, \
            tc.tile_pool(name="qkvp", bufs=2) as qkvp, \
            tc.tile_pool(name="attnp", bufs=5) as attnp, \
            tc.tile_pool(name="smallp", bufs=4) as smallp, \
            tc.tile_pool(name="ahp", bufs=2) as ahp, \
            tc.tile_pool(name="mmp", bufs=3, space="PSUM") as mmp, \
            tc.tile_pool(name="csp", bufs=2, space="PSUM") as csp:

        xT_sb = constp.tile([128, 8, S_], f16, name="xT_sb")
        nc.sync.dma_start(xT_sb[:], xT[:].rearrange("(kt p) s -> p kt s", p=128))

        ones_col = constp.tile([128, 1], f32, name="ones_col")
        nc.vector.memset(ones_col[:], 1.0)

        bias_sb = {}
        for nm, bt in (("q", bq), ("k", bk), ("v", bv)):
            t = constp.tile([128, 4], f32, name=f"b{nm}_sb")
            nc.sync.dma_start(t[:], bt[:].rearrange("(hp p) -> p hp", p=128))
            bias_sb[nm] = t

        import contextlib
        loop_cm = tc.For_i(0, loop, 1) if loop > 1 else contextlib.nullcontext()
        with loop_cm:
            _body(nc, tc, constp, wp, qkvp, attnp, smallp, ahp, mmp, csp,
                  xT_sb, ones_col, bias_sb,
                  wqT, wkT, wvT, attn_o, a_o_r, vt_o, f32, f16, AF)

    nc.compile()
    return nc


def _body(nc, tc, constp, wp, qkvp, attnp, smallp, ahp, mmp, csp,
          xT_sb, ones_col, bias_sb,
          wqT, wkT, wvT, attn_o, a_o_r, vt_o, f32, f16, AF):
        for hp in range(4):
            w_sb = {}
            for nm, wt in (("q", wqT), ("k", wkT), ("v", wvT)):
                t = wp.tile([128, 8, 128], f16, name=f"w{nm}_sb", tag=f"w{nm}")
                nc.sync.dma_start(
                    t[:],
                    wt[:].rearrange("(kt p) m -> p kt m", p=128)[
                        :, :, hp * 128:(hp + 1) * 128
                    ],
                )
                w_sb[nm] = t

            # --- projections: {Q,K,V}t_pair [128, S] (d on partitions) ---
            qkv = {}
            for nm in ("q", "k", "v"):
                dt_o = f32 if nm == "v" else f16
                dst = qkvp.tile([128, S_], dt_o, name=f"{nm}t", tag=f"{nm}t")
                for half in range(2):
                    ps = mmp.tile([128, 1024], f32, name="proj_ps", tag="mm")
                    for nck in range(2):
                        c0 = nck * 512
                        s0 = half * 1024 + c0
                        for kt in range(8):
                            nc.tensor.matmul(
                                ps[:, c0:c0 + 512],
                                w_sb[nm][:, kt, :],
                                xT_sb[:, kt, s0:s0 + 512],
                                start=(kt == 0),
                                stop=(kt == 7),
                            )
                    nc.vector.tensor_scalar_add(
                        dst[:, half * 1024:(half + 1) * 1024],
                        ps[:],
                        bias_sb[nm][:, hp:hp + 1],
                    )
                qkv[nm] = dst

            nc.sync.dma_start(vt_o[hp], qkv["v"][:])

            # --- attention for the two heads of this pair ---
            cs_ps = [
                csp.tile([128, 512], f32, name=f"cs{h}", tag="cs")
                for h in range(2)
            ]

            def emit_colsum(qt, at2):
                for h in range(2):
                    for j in range(4):
                        nc.tensor.matmul(
                            cs_ps[h][32 * j:32 * j + 1, :],
                            ones_col[:],
                            at2[:, h * S_ + j * 512:h * S_ + (j + 1) * 512],
                            start=(qt == 0),
                            stop=(qt == 15),
                            tile_position=(0, 32 * j),
                        )

            pending = []
            for qt in range(16):
                q0 = qt * 128
                eps = {}
                for half in range(2):
                    for h in range(2):
                        eps[(h, half)] = mmp.tile(
                            [128, 1024], f32, name="e_ps", tag="mm")
                    for ncx in range(2):
                        for h in range(2):
                            r0 = h * 64
                            k0 = half * 1024 + ncx * 512
                            nc.tensor.matmul(
                                eps[(h, half)][:, ncx * 512:(ncx + 1) * 512],
                                qkv["q"][r0:r0 + 64, q0:q0 + 128],
                                qkv["k"][r0:r0 + 64, k0:k0 + 512],
                                start=True,
                                stop=True,
                                tile_position=(r0, 0),
                            )

                acc = smallp.tile([128, 4], f32, name="acc", tag="acc")
                at2 = attnp.tile([128, 2 * S_], f32, name="attn_t", tag="attn")
                for h in range(2):
                    for half in range(2):
                        nc.scalar.activation(
                            at2[:, h * S_ + half * 1024:h * S_ + (half + 1) * 1024],
                            eps[(h, half)][:],
                            AF.Exp,
                            scale=SCALE,
                            accum_out=acc[:, half * 2 + h:half * 2 + h + 1],
                        )

                s2 = smallp.tile([128, 2], f32, name="s2", tag="s2")
                nc.vector.tensor_add(s2[:], acc[:, 0:2], acc[:, 2:4])
                r2 = smallp.tile([128, 2], f32, name="r2", tag="r2")
                nc.vector.reciprocal(r2[:], s2[:])

                for h in range(2):
                    nc.vector.tensor_scalar_mul(
                        at2[:, h * S_:(h + 1) * S_],
                        at2[:, h * S_:(h + 1) * S_],
                        r2[:, h:h + 1],
                    )
                nc.sync.dma_start(
                    attn_o[hp * 2:hp * 2 + 2, q0:q0 + 128, :].rearrange(
                        "h p k -> p h k"
                    ),
                    at2[:].rearrange("p (h k) -> p h k", h=2),
                )
                # colsum lags one q-tile so the PE never stalls on the
                # exp->rowsum->reciprocal->normalize chain of this tile.
                pending.append((qt, at2))
                if len(pending) > 2:
                    emit_colsum(*pending.pop(0))
            while pending:
                emit_colsum(*pending.pop(0))

            for h in range(2):
                ah = ahp.tile([128, 512], f32, name="ah", tag="ah")
                for j in range(4):
                    nc.vector.tensor_copy(
                        ah[32 * j:32 * j + 1, :], cs_ps[h][32 * j:32 * j + 1, :]
                    )
                for j in range(4):
                    nc.sync.dma_start(
                        a_o_r[hp * 2 + h][j:j + 1, :], ah[32 * j:32 * j + 1, :]
                    )


def _get_nc():
    if "nc" not in _CACHE:
        _CACHE["nc"] = _build_nc()
    return _CACHE["nc"]


def _numpy_reference(lstm_output, mask, Wq, bq, Wk, bk, Wv, bv, Wo, bo):
    """Exact numpy fallback (used only if mask is not all-ones)."""
    B, S, D = lstm_output.shape
    H, Hd = H_, HD_

    def proj(W, b):
        y = lstm_output @ W.T + b
        return y.reshape(B, S, H, Hd).transpose(0, 2, 1, 3)

    Q, K, V = proj(Wq, bq), proj(Wk, bk), proj(Wv, bv)
    energy = np.einsum("bhqd,bhkd->bhqk", Q, K) * np.float32(1.0 / np.sqrt(Hd))
    m = mask[:, None, None, :]
    energy = np.where(m == 0, np.float32(-10000.0), energy).astype(np.float32)
    e = np.exp(energy - energy.max(axis=-1, keepdims=True))
    attn = e / e.sum(axis=-1, keepdims=True)
    out = np.einsum("bhqk,bhkd->bhqd", attn, V)
    out = out.transpose(0, 2, 1, 3).reshape(B, S, D)
    out = out @ Wo.T + bo
    weighted = out.sum(axis=1)
    return weighted.astype(np.float32), attn.astype(np.float32)


def kernel(lstm_output, mask, Wq, bq, Wk, bk, Wv, bv, Wo, bo):
    lstm_output = np.ascontiguousarray(np.asarray(lstm_output, dtype=np.float32))
    mask = np.asarray(mask)
    Wq, Wk, Wv, Wo = (np.asarray(w, dtype=np.float32) for w in (Wq, Wk, Wv, Wo))
    bq, bk, bv, bo = (np.asarray(b, dtype=np.float32) for b in (bq, bk, bv, bo))

    if not np.all(mask == 1):
        return _numpy_reference(
            lstm_output, mask, Wq, bq, Wk, bk, Wv, bv, Wo, bo
        )

    nc = _get_nc()

    xTs = [np.ascontiguousarray(lstm_output[b].T.astype(np.float16)) for b in range(B_)]
    wqTs = [np.ascontiguousarray(Wq[g * G_:(g + 1) * G_, :].T.astype(np.float16)) for g in range(2)]
    wkTs = [np.ascontiguousarray(Wk[g * G_:(g + 1) * G_, :].T.astype(np.float16)) for g in range(2)]
    wvTs = [np.ascontiguousarray(Wv[g * G_:(g + 1) * G_, :].T.astype(np.float16)) for g in range(2)]

    in_maps = []
    for c in range(NCORES):
        b, g = c // 2, c % 2
        in_maps.append({
            "xT": xTs[b],
            "wqT": wqTs[g],
            "wkT": wkTs[g],
            "wvT": wvTs[g],
            "bq": np.ascontiguousarray(bq[g * G_:(g + 1) * G_]),
            "bk": np.ascontiguousarray(bk[g * G_:(g + 1) * G_]),
            "bv": np.ascontiguousarray(bv[g * G_:(g + 1) * G_]),
        })

    from concourse import bass_utils

    res = bass_utils.run_bass_kernel_spmd(
        nc, in_maps, core_ids=list(range(NCORES))
    )
    outs = res.results

    attn = np.empty((B_, H_, S_, S_), np.float32)
    a_all = np.empty((B_, H_, S_), np.float32)
    vt_all = np.empty((B_, 2, 4, 128, S_), np.float32)
    for c in range(NCORES):
        b, g = c // 2, c % 2
        attn[b, g * 8:(g + 1) * 8] = outs[c]["attn_o"]
        a_all[b, g * 8:(g + 1) * 8] = outs[c]["a_o"]
        vt_all[b, g] = outs[c]["vt_o"]

    # Host-side unshard tail (~17 MFLOP):
    # wsum[b, h, d] = sum_k a[b, h, k] * V[b, k, h*64+d]
    # vt_all[b, g, hp, head*64+d, k] holds V^T for head h = g*8 + 2*hp + head
    vt = vt_all.reshape(B_, 2, 4, 2, 64, S_)          # [b, g, hp, head, d, k]
    vt = vt.reshape(B_, H_, 64, S_)                   # [b, h, d, k]
    wsum = np.einsum("bhdk,bhk->bhd", vt, a_all)      # [b, h, d]
    weighted = wsum.reshape(B_, D_) @ Wo.T + np.float32(S_) * bo
    return weighted.astype(np.float32), attn


if __name__ == "__main__":
    # smoke-build only
    nco = _get_nc()
    print("built ok:", nco)
